# revision 4
# baseline (speedup 1.0000x reference)
# LoRA-MoE QK kernel for 8x Trainium2 NeuronCores (Bass/Tile).
#
# Reference computation:
#   routing = softmax(mean(x[:, 611:-1, :]) @ router_W.T + router_b)   [B, E]
#   base    = x @ W.T + b
#   lora    = einsum('bsd,erd->bser', x, A) -> *B,routing -> [B,S,O] * 2.0
#   out     = base + lora
#
# Sharding: data-parallel over the 8192 tokens (1024/core; each core's tokens
# belong to exactly one batch; a batch spans cores {2b, 2b+1}).  Weights
# replicated, host-prepped (bf16 cast + transpose) so the device only issues
# natural-layout DMAs:
#   xT  [D, 1024] tokens of this core (d-major)
#   wT  [D, O]; afT [D, E*R]; bfT [E*R, O] (2.0 scaling folded in)
# Router runs on host (numpy) and arrives as a per-partition scale svec.
#
# Schedule: the PE starts as soon as the first x k-tile lands.  Per k-tile of
# the contraction dim, phase A runs the 2 LoRA-t matmuls (pt[er,t], 2 PSUM
# banks) plus 6 base-matmul groups of W-panel 0 (tokens 0..767, 6 banks) so
# compute overlaps the x/afT/w0 DMA stream.  Phase A2 finishes panel-0 tokens
# 768..1023 while DVE turns pt into u = t*routing*2 (bf16).  Panels 1..7 then
# run the baseline pipeline: per (panel, token-tile) a 32-deep PSUM group of
# base matmuls closed by one LoRA matmul (u.T @ bfT), bias added on the
# PSUM->SBUF drain, DMA out.

import numpy as np
import ml_dtypes

BF16 = ml_dtypes.bfloat16

B_, S, D, O, E, R = 4, 2048, 4096, 4096, 8, 16
ER = E * R              # 128
TOK = B_ * S            # 8192
NCORES = 8
TPC = TOK // NCORES     # 1024 tokens per core
KT = D // 128           # 32 contraction tiles
NOB = O // 512          # 8 output-column panels
NTT = TPC // 128        # 8 token tiles per core
Q_LO, Q_HI = 611, 2047  # question tokens [611, 2047) within each batch

_CACHE: dict = {}
LAST_RESULTS = None
TRACE = False


def _build_nc():
    import concourse.bacc as bacc
    import concourse.mybir as mybir
    from concourse import tile

    fp32 = mybir.dt.float32
    bf16 = mybir.dt.bfloat16

    nc = bacc.Bacc(
        "TRN2",
        target_bir_lowering=False,
        debug=False,
        num_devices=NCORES,
    )

    xT = nc.dram_tensor("xT", [D, TPC], bf16, kind="ExternalInput")
    wT = nc.dram_tensor("wT", [D, O], bf16, kind="ExternalInput")
    afT = nc.dram_tensor("afT", [D, ER], bf16, kind="ExternalInput")
    bfT = nc.dram_tensor("bfT", [ER, O], bf16, kind="ExternalInput")
    biasrep = nc.dram_tensor("biasrep", [128, O], bf16, kind="ExternalInput")
    svec = nc.dram_tensor("svec", [128, 1], fp32, kind="ExternalInput")
    out = nc.dram_tensor("out", [TPC, O], fp32, kind="ExternalOutput")

    NA = 6  # token-tiles of panel 0 folded into phase A (2 pt banks + 6 = 8)

    with tile.TileContext(nc) as tc:
        with (
            tc.tile_pool(name="const", bufs=1) as const,
            tc.tile_pool(name="xk", bufs=KT) as xpool,
            tc.tile_pool(name="afk", bufs=KT) as afpool,
            tc.tile_pool(name="w", bufs=2 * KT) as wpool,
            tc.tile_pool(name="ot", bufs=4) as otpool,
            tc.tile_pool(name="ps", bufs=8, space="PSUM") as ps_pool,
        ):
            # ---- resident SBUF tensors ----
            bfT_sb = const.tile([128, O], bf16)            # [er, o]
            biasrep_sb = const.tile([128, O], bf16)
            svec_sb = const.tile([128, 1], fp32)
            u_sb = const.tile([128, TPC], bf16)            # [er, t]

            # ---- DMA stream: svec, then per-k (x_k, af_k, w0_k) so compute
            # can chase the stream; bfT/bias afterwards (needed ~55us in) ----
            nc.sync.dma_start(svec_sb[:], svec[:])
            xk = []
            afk = []
            w0 = []
            for k in range(KT):
                xt = xpool.tile([128, TPC], bf16, tag="x")
                nc.sync.dma_start(xt[:], xT[k * 128:(k + 1) * 128, :])
                xk.append(xt)
                af = afpool.tile([128, ER], bf16, tag="af")
                nc.sync.dma_start(af[:], afT[k * 128:(k + 1) * 128, :])
                afk.append(af)
                w_k = wpool.tile([128, 512], bf16, tag="w")
                nc.sync.dma_start(w_k[:], wT[k * 128:(k + 1) * 128, 0:512])
                w0.append(w_k)
            nc.sync.dma_start(bfT_sb[:], bfT[:])
            for kk in range(4):
                nc.sync.dma_start(
                    biasrep_sb[:, kk * 1024:(kk + 1) * 1024],
                    biasrep[:, kk * 1024:(kk + 1) * 1024],
                )

            # ---- phase A: per k, LoRA-t accumulation + 6 panel-0 groups ----
            pt0 = ps_pool.tile([128, 512], fp32, tag="ps")
            pt1 = ps_pool.tile([128, 512], fp32, tag="ps")
            poA = [
                ps_pool.tile([128, 512], fp32, name=f"poA{i}", tag="ps") for i in range(NA)
            ]
            for k in range(KT):
                nc.tensor.matmul(
                    pt0[:], afk[k][:], xk[k][:, 0:512],
                    start=(k == 0), stop=(k == KT - 1),
                )
                nc.tensor.matmul(
                    pt1[:], afk[k][:], xk[k][:, 512:1024],
                    start=(k == 0), stop=(k == KT - 1),
                )
                for tt in range(NA):
                    nc.tensor.matmul(
                        poA[tt][:],
                        xk[k][:, tt * 128:(tt + 1) * 128],
                        w0[k][:],
                        start=(k == 0), stop=False,
                    )

            # ---- u = t * routing (per-partition scalar), bf16, on DVE ----
            nc.vector.tensor_scalar_mul(u_sb[:, 0:512], pt0[:], svec_sb[:, 0:1])
            nc.vector.tensor_scalar_mul(u_sb[:, 512:1024], pt1[:], svec_sb[:, 0:1])

            def close_and_drain(po, tt, ob):
                nc.tensor.matmul(
                    po[:],
                    u_sb[:, tt * 128:(tt + 1) * 128],
                    bfT_sb[:, ob * 512:(ob + 1) * 512],
                    start=False, stop=True,
                )
                ot = otpool.tile([128, 512], fp32)
                nc.vector.tensor_add(
                    ot[:], po[:], biasrep_sb[:, ob * 512:(ob + 1) * 512]
                )
                nc.sync.dma_start(
                    out[tt * 128:(tt + 1) * 128, ob * 512:(ob + 1) * 512],
                    ot[:],
                )

            # ---- phase A2: panel-0 token tiles 6,7 (reuse freed pt banks);
            # keeps the PE busy while DVE produces u ----
            poB = []
            for tt in range(NA, NTT):
                po = ps_pool.tile([128, 512], fp32, tag="ps")
                poB.append(po)
                for k in range(KT):
                    nc.tensor.matmul(
                        po[:],
                        xk[k][:, tt * 128:(tt + 1) * 128],
                        w0[k][:],
                        start=(k == 0), stop=False,
                    )
            for tt in range(NA):
                close_and_drain(poA[tt], tt, 0)
            for i, tt in enumerate(range(NA, NTT)):
                close_and_drain(poB[i], tt, 0)

            # ---- panels 1..7: double-buffered W stream, 33-matmul groups ----
            for ob in range(1, NOB):
                wt = []
                for k in range(KT):
                    w_k = wpool.tile([128, 512], bf16, tag="w")
                    nc.sync.dma_start(
                        w_k[:],
                        wT[k * 128:(k + 1) * 128, ob * 512:(ob + 1) * 512],
                    )
                    wt.append(w_k)
                for tt in range(NTT):
                    po = ps_pool.tile([128, 512], fp32, tag="ps")
                    for k in range(KT):
                        nc.tensor.matmul(
                            po[:],
                            xk[k][:, tt * 128:(tt + 1) * 128],
                            wt[k][:],
                            start=(k == 0), stop=False,
                        )
                    close_and_drain(po, tt, ob)

    nc.compile()
    return nc


def _host_prep(x, W, b, A, B, router_W, router_b):
    xf = np.ascontiguousarray(x, dtype=np.float32).reshape(TOK, D)
    xT_bf = xf.T.astype(BF16)                       # [D, TOK]
    wT_bf = W.T.astype(BF16)                        # [D, O]
    afT_bf = A.reshape(ER, D).T.astype(BF16)        # [D, ER]
    bfT_bf = (2.0 * np.transpose(B, (0, 2, 1)).reshape(ER, O)).astype(BF16)
    bias_bf = np.ascontiguousarray(
        np.broadcast_to(b.astype(BF16)[None, :], (128, O))
    )
    # router on host (numpy, float64 — exact vs bf16 device noise)
    xq = np.asarray(x, np.float64)[:, Q_LO:Q_HI, :]
    q = xq.mean(axis=1)
    logits = q @ np.asarray(router_W, np.float64).T + np.asarray(router_b, np.float64)
    ex = np.exp(logits - logits.max(-1, keepdims=True))
    routing = ex / ex.sum(-1, keepdims=True)          # [B, E]

    shards = [
        np.ascontiguousarray(xT_bf[:, c * TPC:(c + 1) * TPC]) for c in range(NCORES)
    ]
    in_maps = []
    for c in range(NCORES):
        sv = np.repeat(routing[c // 2].astype(np.float32), R).reshape(128, 1)
        in_maps.append({
            "xT": shards[c],
            "wT": wT_bf,
            "afT": afT_bf,
            "bfT": bfT_bf,
            "biasrep": bias_bf,
            "svec": np.ascontiguousarray(sv),
        })
    return in_maps


def kernel(x, W, b, A, B, router_W, router_b):
    global LAST_RESULTS
    from concourse.bass_utils import run_bass_kernel_spmd

    if "nc" not in _CACHE:
        _CACHE["nc"] = _build_nc()
    nc = _CACHE["nc"]

    in_maps = _host_prep(x, W, b, A, B, router_W, router_b)

    kwargs = {}
    if TRACE:
        kwargs.update(trace=True, trace_cores=list(range(NCORES)))
    res = run_bass_kernel_spmd(nc, in_maps, core_ids=list(range(NCORES)), **kwargs)
    LAST_RESULTS = res

    shards = [res.results[c]["out"] for c in range(NCORES)]
    return np.concatenate(shards, axis=0).reshape(B_, S, O).astype(np.float32)


# revision 5
# speedup vs baseline: 1.1721x; 1.1721x over previous
# LoRA-MoE QK kernel for 8x Trainium2 NeuronCores (Bass/Tile).
#
# Reference computation:
#   routing = softmax(mean(x[:, 611:-1, :]) @ router_W.T + router_b)   [B, E]
#   base    = x @ W.T + b
#   lora    = einsum('bsd,erd->bser', x, A) -> *B,routing -> [B,S,O] * 2.0
#   out     = base + lora
#
# Sharding: data-parallel over the 8192 tokens (1024/core; each core's tokens
# belong to exactly one batch; a batch spans cores {2b, 2b+1}).  Weights
# replicated, host-prepped (bf16 cast + transpose) so the device only issues
# natural-layout DMAs:
#   xT  [D, 1024] tokens of this core (d-major)
#   wT  [D, O]; afT [D, E*R]; bfT [E*R, O] (2.0 scaling folded in)
# Router runs on host (numpy) and arrives as a per-partition scale svec.
#
# Schedule: the PE starts as soon as the first x k-tile lands.  Per k-tile of
# the contraction dim, phase A runs the 2 LoRA-t matmuls (pt[er,t], 2 PSUM
# banks) plus 6 base-matmul groups of W-panel 0 (tokens 0..767, 6 banks) so
# compute overlaps the x/afT/w0 DMA stream.  Phase A2 finishes panel-0 tokens
# 768..1023 while DVE turns pt into u = t*routing*2 (bf16).  Panels 1..7 then
# run the baseline pipeline: per (panel, token-tile) a 32-deep PSUM group of
# base matmuls closed by one LoRA matmul (u.T @ bfT), bias added on the
# PSUM->SBUF drain, DMA out.

import numpy as np
import ml_dtypes

BF16 = ml_dtypes.bfloat16

B_, S, D, O, E, R = 4, 2048, 4096, 4096, 8, 16
ER = E * R              # 128
TOK = B_ * S            # 8192
NCORES = 8
TPC = TOK // NCORES     # 1024 tokens per core
KT = D // 128           # 32 contraction tiles
NOB = O // 512          # 8 output-column panels
NTT = TPC // 128        # 8 token tiles per core
Q_LO, Q_HI = 611, 2047  # question tokens [611, 2047) within each batch

_CACHE: dict = {}
LAST_RESULTS = None
TRACE = False


def _build_nc():
    import concourse.bacc as bacc
    import concourse.mybir as mybir
    from concourse import tile

    fp32 = mybir.dt.float32
    bf16 = mybir.dt.bfloat16

    nc = bacc.Bacc(
        "TRN2",
        target_bir_lowering=False,
        debug=False,
        num_devices=NCORES,
    )

    xT = nc.dram_tensor("xT", [D, TPC], bf16, kind="ExternalInput")
    wT = nc.dram_tensor("wT", [D, O], bf16, kind="ExternalInput")
    afT = nc.dram_tensor("afT", [128, KT * ER], bf16, kind="ExternalInput")
    bfT = nc.dram_tensor("bfT", [ER, O], bf16, kind="ExternalInput")
    biasrep = nc.dram_tensor("biasrep", [128, O], bf16, kind="ExternalInput")
    svec = nc.dram_tensor("svec", [128, 1], fp32, kind="ExternalInput")
    out = nc.dram_tensor("out", [TPC, O], fp32, kind="ExternalOutput")

    NA = 6  # token-tiles of panel 0 folded into phase A (2 pt banks + 6 = 8)

    with tile.TileContext(nc) as tc:
        with (
            tc.tile_pool(name="const", bufs=1) as const,
            tc.tile_pool(name="xk", bufs=KT) as xpool,
            tc.tile_pool(name="w", bufs=2 * KT) as wpool,
            tc.tile_pool(name="ot", bufs=4) as otpool,
            tc.tile_pool(name="ps", bufs=8, space="PSUM") as ps_pool,
        ):
            # ---- resident SBUF tensors ----
            bfT_sb = const.tile([128, O], bf16)            # [er, o]
            af_sb = const.tile([128, KT * ER], bf16)       # [d%128, (k er)]
            biasrep_sb = const.tile([128, O], bf16)
            svec_sb = const.tile([128, 1], fp32)
            u_sb = const.tile([128, TPC], bf16)            # [er, t]

            # ---- DMA stream: svec, then per-k (x_k, af_k, w0_k) so compute
            # can chase the stream; bfT/bias afterwards (needed ~55us in) ----
            nc.sync.dma_start(svec_sb[:], svec[:])
            xk = []
            w0 = []
            for k in range(KT):
                xt = xpool.tile([128, TPC], bf16, tag="x")
                # split the first tiles by partition for low latency; whole-tile
                # dispatches afterwards (sync-queue dispatch is ~600ns each)
                nx = 4 if k < 2 else 1
                for q in range(nx):
                    pp = 128 // nx
                    nc.sync.dma_start(
                        xt[q * pp:(q + 1) * pp, :],
                        xT[k * 128 + q * pp:k * 128 + (q + 1) * pp, :],
                    )
                xk.append(xt)
                w_k = wpool.tile([128, 512], bf16, tag="w")
                nw = 2 if k < 4 else 1
                for q in range(nw):
                    pp = 128 // nw
                    nc.sync.dma_start(
                        w_k[q * pp:(q + 1) * pp, :],
                        wT[k * 128 + q * pp:k * 128 + (q + 1) * pp, 0:512],
                    )
                w0.append(w_k)
                if k == 1:
                    for q in range(4):
                        nc.sync.dma_start(
                            af_sb[q * 32:(q + 1) * 32, :],
                            afT[q * 32:(q + 1) * 32, :],
                        )
                if k == 8:
                    for q in range(4):
                        nc.sync.dma_start(
                            bfT_sb[q * 32:(q + 1) * 32, :],
                            bfT[q * 32:(q + 1) * 32, :],
                        )
                if k == 16:
                    for q in range(2):
                        nc.sync.dma_start(
                            biasrep_sb[q * 64:(q + 1) * 64, :],
                            biasrep[q * 64:(q + 1) * 64, :],
                        )

            # ---- phase A: per k, LoRA-t accumulation + 6 panel-0 groups ----
            pt0 = ps_pool.tile([128, 512], fp32, tag="ps")
            pt1 = ps_pool.tile([128, 512], fp32, tag="ps")
            poA = [
                ps_pool.tile([128, 512], fp32, name=f"poA{i}", tag="ps") for i in range(NA)
            ]
            LAG = 8

            def pt_mms(kk):
                nc.tensor.matmul(
                    pt0[:], af_sb[:, kk * ER:(kk + 1) * ER], xk[kk][:, 0:512],
                    start=(kk == 0), stop=(kk == KT - 1),
                )
                nc.tensor.matmul(
                    pt1[:], af_sb[:, kk * ER:(kk + 1) * ER], xk[kk][:, 512:1024],
                    start=(kk == 0), stop=(kk == KT - 1),
                )

            for k in range(KT):
                for tt in range(NA):
                    nc.tensor.matmul(
                        poA[tt][:],
                        xk[k][:, tt * 128:(tt + 1) * 128],
                        w0[k][:],
                        start=(k == 0), stop=False,
                    )
                if k >= LAG:
                    pt_mms(k - LAG)
            for kk in range(KT - LAG, KT):
                pt_mms(kk)

            # ---- u = t * routing (per-partition scalar), bf16, on DVE ----
            nc.vector.tensor_scalar_mul(u_sb[:, 0:512], pt0[:], svec_sb[:, 0:1])
            nc.vector.tensor_scalar_mul(u_sb[:, 512:1024], pt1[:], svec_sb[:, 0:1])

            def close_and_drain(po, tt, ob):
                nc.tensor.matmul(
                    po[:],
                    u_sb[:, tt * 128:(tt + 1) * 128],
                    bfT_sb[:, ob * 512:(ob + 1) * 512],
                    start=False, stop=True,
                )
                ot = otpool.tile([128, 512], fp32)
                nc.vector.tensor_add(
                    ot[:], po[:], biasrep_sb[:, ob * 512:(ob + 1) * 512]
                )
                for q in range(2):
                    nc.sync.dma_start(
                        out[tt * 128 + q * 64:tt * 128 + (q + 1) * 64,
                            ob * 512:(ob + 1) * 512],
                        ot[q * 64:(q + 1) * 64, :],
                    )

            # ---- phase A2: panel-0 token tiles 6,7 (reuse freed pt banks);
            # keeps the PE busy while DVE produces u ----
            poB = []
            for tt in range(NA, NTT):
                po = ps_pool.tile([128, 512], fp32, tag="ps")
                poB.append(po)
                for k in range(KT):
                    nc.tensor.matmul(
                        po[:],
                        xk[k][:, tt * 128:(tt + 1) * 128],
                        w0[k][:],
                        start=(k == 0), stop=False,
                    )
            for tt in range(NA):
                close_and_drain(poA[tt], tt, 0)
            for i, tt in enumerate(range(NA, NTT)):
                close_and_drain(poB[i], tt, 0)

            # ---- panels 1..7: double-buffered W stream, 33-matmul groups ----
            for ob in range(1, NOB):
                wt = []
                for k in range(KT):
                    w_k = wpool.tile([128, 512], bf16, tag="w")
                    nc.sync.dma_start(
                        w_k[:],
                        wT[k * 128:(k + 1) * 128, ob * 512:(ob + 1) * 512],
                    )
                    wt.append(w_k)
                for tt in range(NTT):
                    po = ps_pool.tile([128, 512], fp32, tag="ps")
                    for k in range(KT):
                        nc.tensor.matmul(
                            po[:],
                            xk[k][:, tt * 128:(tt + 1) * 128],
                            wt[k][:],
                            start=(k == 0), stop=False,
                        )
                    close_and_drain(po, tt, ob)

    nc.compile()
    return nc


def _host_prep(x, W, b, A, B, router_W, router_b):
    xf = np.ascontiguousarray(x, dtype=np.float32).reshape(TOK, D)
    xT_bf = xf.T.astype(BF16)                       # [D, TOK]
    wT_bf = W.T.astype(BF16)                        # [D, O]
    af_t = A.reshape(ER, D).T.astype(BF16)          # [D, ER]
    afT_bf = np.ascontiguousarray(
        af_t.reshape(KT, 128, ER).transpose(1, 0, 2).reshape(128, KT * ER)
    )                                               # [128, (k er)]
    bfT_bf = (2.0 * np.transpose(B, (0, 2, 1)).reshape(ER, O)).astype(BF16)
    bias_bf = np.ascontiguousarray(
        np.broadcast_to(b.astype(BF16)[None, :], (128, O))
    )
    # router on host (numpy, float64 — exact vs bf16 device noise)
    xq = np.asarray(x, np.float64)[:, Q_LO:Q_HI, :]
    q = xq.mean(axis=1)
    logits = q @ np.asarray(router_W, np.float64).T + np.asarray(router_b, np.float64)
    ex = np.exp(logits - logits.max(-1, keepdims=True))
    routing = ex / ex.sum(-1, keepdims=True)          # [B, E]

    shards = [
        np.ascontiguousarray(xT_bf[:, c * TPC:(c + 1) * TPC]) for c in range(NCORES)
    ]
    in_maps = []
    for c in range(NCORES):
        sv = np.repeat(routing[c // 2].astype(np.float32), R).reshape(128, 1)
        in_maps.append({
            "xT": shards[c],
            "wT": wT_bf,
            "afT": afT_bf,
            "bfT": bfT_bf,
            "biasrep": bias_bf,
            "svec": np.ascontiguousarray(sv),
        })
    return in_maps


def kernel(x, W, b, A, B, router_W, router_b):
    global LAST_RESULTS
    from concourse.bass_utils import run_bass_kernel_spmd

    if "nc" not in _CACHE:
        _CACHE["nc"] = _build_nc()
    nc = _CACHE["nc"]

    in_maps = _host_prep(x, W, b, A, B, router_W, router_b)

    kwargs = {}
    if TRACE:
        kwargs.update(trace=True, trace_cores=list(range(NCORES)))
    res = run_bass_kernel_spmd(nc, in_maps, core_ids=list(range(NCORES)), **kwargs)
    LAST_RESULTS = res

    shards = [res.results[c]["out"] for c in range(NCORES)]
    return np.concatenate(shards, axis=0).reshape(B_, S, O).astype(np.float32)
